# revision 10
# baseline (speedup 1.0000x reference)
"""ChamferLoss Trainium2 kernel (8 NeuronCores, bass/Tile) — banded version.

pred, target: [2, 16384, 3] fp32 -> scalar fp32
  d[b,n,m] = ||pred[b,n] - target[b,m]||
  out = mean(min_m d) + mean(min_n d)

Strategy: both point sets are sorted by norm on the host. Since
d(p,t) >= | ||p|| - ||t|| |, the nearest neighbour of a pred is (almost
always) close in *norm rank*, so each 128-pred block only computes d^2
against a 2048-wide window of rank-aligned targets (8x less work than
brute force). Exactness is restored on the host: for every point the
norm-gap to the first UNSEEN target/pred rank lower-bounds all unseen
distances; points whose windowed min exceeds that bound are recomputed
exactly in numpy (a few thousand on this distribution).

Sharding: core c = (batch b=c//4, pred-quarter q=c%4): 4096 sorted preds
x a 6144-wide sorted-target slice (lo = 4096q-960, indices clamped at the
array ends so edge windows degrade gracefully into duplicated targets).
Per core:
  - PE: d^2 tiles via one K=128 bf16 matmul per 512 targets. The 30
    augmented contraction rows (three-term bf16 splits of the coordinates
    and squared norms, so d^2 = p2 + t2 - 2 p.t accumulates in fp32 PSUM
    at ~fp32 accuracy) are replicated 4x; the 4x-scaled sum is undone by
    the ScalarE convert's scale.
  - ScalarE: PSUM fp32 -> SBUF fp16 conversion (scaled by BOOST/NREP).
  - VectorE: per block, one tensor_tensor min folds the 2048-wide window
    into the backward accumulator bacc[128, 6144], and one
    tensor_tensor_reduce (pairwise min of the window halves + min
    accumulator) produces the block's forward min column directly.
  - Tail: PE transposes bacc in 128-col chunks (4 per PSUM tile); VectorE
    reduce_min over [128, 4, 128] -> per-position min.
Host: maps window positions back to target ranks, mins across cores,
applies the gap-condition patch-up, then sqrt + means (O(N) work).
"""

import ml_dtypes
import numpy as np

import concourse.bass as bass
import concourse.tile as tile
from concourse import mybir

F32 = mybir.dt.float32
F16 = mybir.dt.float16
BF16 = mybir.dt.bfloat16

B = 2
N = 16384          # preds per batch
M = 16384          # targets per batch
NQ = N // 4        # preds per core
KA = 30            # base augmented contraction depth
NREP = 4           # replication count (30*4 = 120 <= 128)
K = 128            # padded contraction depth
NB = NQ // 128     # pred blocks per core (32)
W = 2048           # target window per pred block
WSPAN = 6144       # per-core target slice width (covers 128*31 + 2048 = 6016)
WCOV = 128 * (NB - 1) + W   # = 6016, highest window end within the slice
LO_OFF = -960      # slice start = 4096*q + LO_OFF (clamped indexing)
MM_N = 512         # matmul free dim (one PSUM bank)
N_CORES = 8
BOOST = 64.0       # pre-conversion scale: keeps tiny d^2 out of fp16
                   # subnormals (max d^2 ~ 300 * 64 still << fp16 max)
FBIG = 60000.0     # "+inf" for f16 min accumulators
FWD_MODE = "ts_accum"  # "ts_accum" | "ttr" | "fold" forward-min strategy


# --------------------------------------------------------------------------
# Workaround: this walrus build accepts at most one sync-wait command per
# instruction. Hoist extra waits onto same-engine NoOps placed just before.
# --------------------------------------------------------------------------

def _split_sync_waits(nc):
    counter = 0
    for block in nc.m.functions[0].blocks:
        insts = block.instructions
        out = []
        changed = False
        for inst in insts:
            si = inst.sync_info
            if si is not None and si.on_wait and len(si.on_wait) > 1:
                waits = list(si.on_wait)
                for w in waits[:-1]:
                    counter += 1
                    out.append(
                        mybir.InstNoOp(
                            name=f"waitnop-{counter}",
                            engine=inst.engine,
                            sync_info=mybir.SyncInfo(on_wait=[w], on_update=[]),
                        )
                    )
                si.on_wait = waits[-1:]
                changed = True
            out.append(inst)
        if changed:
            block.instructions = out


def _patch_bass():
    if getattr(bass.Bass, "_split_waits_patched", False):
        return
    orig = bass.Bass.to_json_bytes

    def to_json_bytes(self, *a, **kw):
        _split_sync_waits(self)
        # populate .instr bytes for InstISA subclasses (tensor_tensor_reduce
        # etc.) — raw bass doesn't run this pass and walrus then fails with
        # "ISA wrong length" on the empty instr field
        mybir.codegen_inst_isa_subclasses(self)
        return orig(self, *a, **kw)

    bass.Bass.to_json_bytes = to_json_bytes
    bass.Bass._split_waits_patched = True


# --------------------------------------------------------------------------
# Kernel builder
# --------------------------------------------------------------------------

def build_kernel(n_loop: int = 0):
    """n_loop=0: production straight-line kernel. n_loop>0: wrap the main
    (idempotent) compute in a For_i loop for slope timing."""
    _patch_bass()
    nc = bass.Bass()
    paug_d = nc.dram_tensor("paug", [K, NQ], BF16, kind="ExternalInput")
    taug_d = nc.dram_tensor("taug", [K, WSPAN], BF16, kind="ExternalInput")
    fmin_d = nc.dram_tensor("fmin", [128, NB], F16, kind="ExternalOutput")
    bmin_d = nc.dram_tensor("bmin", [128, WSPAN // 128], F32,
                            kind="ExternalOutput")

    CVT_SCALE = BOOST / NREP

    with tile.TileContext(nc) as tc:
        with (
            tc.tile_pool(name="singles", bufs=1) as singles,
            tc.tile_pool(name="work", bufs=3) as work,
        ):
            paug = singles.tile([K, NQ], BF16)
            taug = singles.tile([K, WSPAN], BF16)
            bacc = singles.tile([128, WSPAN], F16)
            fmin_sb = singles.tile([128, NB], F16)
            bmin_sb = singles.tile([128, WSPAN // 128], F32)

            nc.sync.dma_start(out=paug[:], in_=paug_d[:])
            for g in range(3):
                nc.sync.dma_start(
                    out=taug[:, g * 2048:(g + 1) * 2048],
                    in_=taug_d[:, g * 2048:(g + 1) * 2048],
                )

            ident = singles.tile([128, 128], F16)
            nc.gpsimd.memset(ident[:], 0.0)
            nc.gpsimd.affine_select(
                out=ident[:],
                in_=ident[:],
                compare_op=mybir.AluOpType.not_equal,
                fill=1.0,
                base=0,
                pattern=[[-1, 128]],
                channel_multiplier=1,
            )
            # backward accumulator starts at "+inf" (outside the timed loop;
            # the min-accumulation below is idempotent across loop iters)
            nc.vector.memset(bacc[:], FBIG)

            with tc.tile_pool(name="psum", bufs=2, space="PSUM") as psum:
                def tail_fold(t4):
                    # backward partition fold for 4 x 128 cols of bacc
                    tp = psum.tile([128, 512], F16, name=f"tp{t4}", tag="d2")
                    for u in range(4):
                        t = t4 * 4 + u
                        nc.tensor.transpose(
                            tp[:, u * 128:(u + 1) * 128],
                            bacc[:, t * 128:(t + 1) * 128],
                            ident[:],
                        )
                    nc.vector.tensor_reduce(
                        out=bmin_sb[:, t4 * 4:(t4 + 1) * 4],
                        in_=tp[:].rearrange("p (u f) -> p u f", u=4),
                        axis=mybir.AxisListType.X,
                        op=mybir.AluOpType.min,
                    )

                def main_compute():
                    for nb in range(NB):
                        lhsT = paug[:, nb * 128:(nb + 1) * 128]
                        d2 = psum.tile([128, W], F32, name=f"d2_{nb}",
                                       tag="d2")
                        for j in range(W // MM_N):
                            nc.tensor.matmul(
                                d2[:, j * MM_N:(j + 1) * MM_N],
                                lhsT,
                                taug[:, nb * 128 + j * MM_N:
                                     nb * 128 + (j + 1) * MM_N],
                                start=True,
                                stop=True,
                            )
                        cvt = work.tile([128, W], F16, name=f"cvt{nb}",
                                        tag="cvt")
                        nc.scalar.activation(
                            out=cvt[:], in_=d2[:],
                            func=mybir.ActivationFunctionType.Copy,
                            scale=CVT_SCALE,
                        )
                        # backward: fold the window into the running min
                        nc.vector.tensor_tensor(
                            out=bacc[:, nb * 128:nb * 128 + W],
                            in0=bacc[:, nb * 128:nb * 128 + W],
                            in1=cvt[:],
                            op=mybir.AluOpType.min,
                        )
                        # forward: reduce the window to this block's
                        # forward-min column
                        if FWD_MODE == "ts_accum":
                            # single-src tensor_scalar, min-reduce accumulator
                            fold = work.tile([128, W], F16,
                                             name=f"fold{nb}", tag="fold")
                            nc.vector.tensor_scalar(
                                out=fold[:],
                                in0=cvt[:],
                                scalar1=FBIG,
                                scalar2=None,
                                op0=mybir.AluOpType.min,
                                op1=mybir.AluOpType.min,
                                accum_out=fmin_sb[:, nb:nb + 1],
                            )
                        elif FWD_MODE == "ttr":
                            fold = work.tile([128, W // 2], F16,
                                             name=f"fold{nb}", tag="fold")
                            nc.vector.tensor_tensor_reduce(
                                out=fold[:],
                                in0=cvt[:, 0:W // 2],
                                in1=cvt[:, W // 2:W],
                                scale=1.0,
                                scalar=FBIG,
                                op0=mybir.AluOpType.min,
                                op1=mybir.AluOpType.min,
                                accum_out=fmin_sb[:, nb:nb + 1],
                            )
                        else:
                            fold = work.tile([128, W // 2], F16,
                                             name=f"fold{nb}", tag="fold")
                            nc.vector.tensor_tensor(
                                out=fold[:],
                                in0=cvt[:, 0:W // 2],
                                in1=cvt[:, W // 2:W],
                                op=mybir.AluOpType.min,
                            )
                            nc.vector.tensor_tensor(
                                out=fold[:, 0:W // 4],
                                in0=fold[:, 0:W // 4],
                                in1=fold[:, W // 4:W // 2],
                                op=mybir.AluOpType.min,
                            )
                            nc.vector.tensor_reduce(
                                out=fmin_sb[:, nb:nb + 1],
                                in_=fold[:, 0:W // 4],
                                axis=mybir.AxisListType.X,
                                op=mybir.AluOpType.min,
                            )
                        # bacc cols [0, 128*nb) are final once block nb is
                        # done; fold them now so the tail overlaps compute
                        if nb >= 1 and nb * 128 % 512 == 0:
                            tail_fold(nb // 4 - 1)
                    for t4 in range(NB // 4 - 1, WSPAN // 512):
                        tail_fold(t4)

                if n_loop:
                    with tc.For_i(0, n_loop, 1):
                        main_compute()
                else:
                    main_compute()

            nc.sync.dma_start(out=fmin_d[:], in_=fmin_sb[:])
            nc.sync.dma_start(out=bmin_d[:], in_=bmin_sb[:])
    return nc


# --------------------------------------------------------------------------
# Host-side prep: augmented coordinate matrices. Each fp32 value is split
# into three bf16 terms (h + m + l reproduces the fp32 value to ~2^-24), so
# the expanded d^2 = p2 + t2 - 2 p.t keeps ~fp32-level absolute accuracy
# even for near-duplicate clouds where d^2 << |p|^2 (heavy cancellation).
# Cross terms keep the 8 products with magnitude >= 2^-25 (drop l*l);
# 30 rows total, replicated NREP=4 times and zero-padded to K=128.
# --------------------------------------------------------------------------

def _bf16(x):
    return x.astype(ml_dtypes.bfloat16)


def _split3(x):
    """fp32 array -> three bf16 arrays whose sum reproduces x to ~2^-24."""
    h = _bf16(x)
    r1 = x - h.astype(np.float32)
    m = _bf16(r1)
    l = _bf16(r1 - m.astype(np.float32))
    return h, m, l


def _aug_parts(coords):
    c = coords.astype(np.float32).T  # [3, n]
    n2 = c[0] * c[0] + c[1] * c[1] + c[2] * c[2]  # fp32, matches reference
    return _split3(c), _split3(n2)


def _replicate(base):
    out = np.zeros((K, base.shape[1]), dtype=ml_dtypes.bfloat16)
    for r in range(NREP):
        out[r * KA:(r + 1) * KA] = base
    return out


# (pred_term, target_term) index pairs for the 8 kept cross products
_CROSS = [(0, 0), (0, 1), (0, 2), (1, 0), (1, 1), (1, 2), (2, 0), (2, 1)]


def _aug_pred(coords):
    (ch, cm, cl), (n2h, n2m, n2l) = _aug_parts(coords)
    terms = [ch, cm, cl]
    base = np.zeros((KA, coords.shape[0]), dtype=ml_dtypes.bfloat16)
    for i, (pi, _) in enumerate(_CROSS):
        base[3 * i:3 * i + 3] = _bf16(-2.0 * terms[pi].astype(np.float32))
    base[24] = n2h
    base[25] = n2m
    base[26] = n2l
    base[27:30] = 1.0
    return _replicate(base)


def _aug_target(coords):
    (ch, cm, cl), (n2h, n2m, n2l) = _aug_parts(coords)
    terms = [ch, cm, cl]
    base = np.zeros((KA, coords.shape[0]), dtype=ml_dtypes.bfloat16)
    for i, (_, ti) in enumerate(_CROSS):
        base[3 * i:3 * i + 3] = terms[ti]
    base[24:27] = 1.0
    base[27] = n2h
    base[28] = n2m
    base[29] = n2l
    return _replicate(base)


def _sorted_views(pred, target):
    """Per batch: norm-sorted points + norms (the kernel's working order)."""
    meta = []
    for b in range(B):
        p = np.asarray(pred[b], np.float32)
        t = np.asarray(target[b], np.float32)
        pn = np.sqrt(np.sum(p * p, axis=1))
        tn = np.sqrt(np.sum(t * t, axis=1))
        po = np.argsort(pn, kind="stable")
        to = np.argsort(tn, kind="stable")
        meta.append({
            "ps": p[po], "ts": t[to],
            "psn": pn[po], "tsn": tn[to],
        })
    return meta


def make_in_maps(pred, target):
    meta = _sorted_views(pred, target)
    in_maps = []
    for b in range(B):
        ps, ts = meta[b]["ps"], meta[b]["ts"]
        taug_full = _aug_target(ts)
        for q in range(4):
            lo = 4096 * q + LO_OFF
            idx = np.clip(np.arange(lo, lo + WSPAN), 0, M - 1)
            in_maps.append({
                "paug": _aug_pred(ps[q * NQ:(q + 1) * NQ]),
                "taug": np.ascontiguousarray(taug_full[:, idx]),
            })
    # core order: c = b*4 + q
    return in_maps


# --------------------------------------------------------------------------
# Host post: map window positions back to ranks, min across cores, verify
# the norm-gap bound, recompute flagged points exactly, then sqrt + mean.
# --------------------------------------------------------------------------

def _exact_rows(pts, others):
    """Exact min distance from each row of pts[V,3] to others[M,3] (fp32,
    same formula as the reference)."""
    p2 = np.sum(pts * pts, axis=1, dtype=np.float32)[:, None]
    t2 = np.sum(others * others, axis=1, dtype=np.float32)[None, :]
    d2 = p2 + t2 - 2.0 * (pts @ others.T)
    return np.sqrt(np.maximum(d2.min(axis=1), 0.0))


def postprocess(results, meta):
    total = np.float64(0.0)
    pos = np.arange(WSPAN)
    i_min = np.maximum(0, -(-(pos - (W - 1)) // 128))   # ceil((pos-2047)/128)
    i_max = np.minimum(NB - 1, pos // 128)
    covered_pos = (i_min <= i_max) & (pos < WCOV)
    for b in range(B):
        mb = meta[b]
        psn, tsn = mb["psn"], mb["tsn"]
        ps, ts = mb["ps"], mb["ts"]

        fmin = np.full(N, np.inf, np.float32)
        bmin = np.full(M, np.inf, np.float32)
        pLo = np.full(M, N, np.int64)
        pHi = np.full(M, 0, np.int64)
        covL = np.full(N, 0, np.int64)
        covR = np.full(N, 0, np.int64)
        for q in range(4):
            r = results[b * 4 + q]
            lo = 4096 * q + LO_OFF
            # forward: fmin_sb[p, i] = window min for pred rank 4096q+128i+p
            f = np.asarray(r["fmin"]).T.reshape(-1)       # rank = 128*i + p
            fmin[q * NQ:(q + 1) * NQ] = f
            blk = np.arange(NQ) // 128
            covL[q * NQ:(q + 1) * NQ] = np.clip(lo + 128 * blk, 0, M)
            covR[q * NQ:(q + 1) * NQ] = np.clip(lo + 128 * blk + W, 0, M)
            # backward: bmin_sb[p, t] = min over preds for position 128t+p
            bm = np.asarray(r["bmin"]).T.reshape(-1)      # position
            ranks = np.clip(lo + pos, 0, M - 1)
            sel = covered_pos
            np.minimum.at(bmin, ranks[sel], bm[sel])
            np.minimum.at(pLo, ranks[sel], q * NQ + 128 * i_min[sel])
            np.maximum.at(pHi, ranks[sel], q * NQ + 128 * i_max[sel] + 128)

        fwd = np.sqrt(np.maximum(fmin * np.float32(1.0 / BOOST), 0.0,
                                 dtype=np.float32))
        bwd = np.sqrt(np.maximum(bmin * np.float32(1.0 / BOOST), 0.0,
                                 dtype=np.float32))

        # gap condition (forward): unseen targets are all below covL or at/
        # above covR in rank; their distance is >= the norm gap.
        gapL = np.where(covL > 0, psn - tsn[np.maximum(covL - 1, 0)], np.inf)
        gapR = np.where(covR < M, tsn[np.minimum(covR, M - 1)] - psn, np.inf)
        gap = np.minimum(np.maximum(gapL, 0.0), np.maximum(gapR, 0.0))
        bad = fwd * np.float32(1.002) + np.float32(1e-6) > gap
        if np.any(bad):
            fwd[bad] = _exact_rows(ps[bad], ts)

        # gap condition (backward)
        gapL = np.where(pLo > 0, tsn - psn[np.maximum(pLo - 1, 0)], np.inf)
        gapR = np.where(pHi < N, psn[np.minimum(pHi, N - 1)] - tsn, np.inf)
        gap = np.minimum(np.maximum(gapL, 0.0), np.maximum(gapR, 0.0))
        bad = bwd * np.float32(1.002) + np.float32(1e-6) > gap
        if np.any(bad):
            bwd[bad] = _exact_rows(ts[bad], ps)

        total += (fwd.mean(dtype=np.float64) + bwd.mean(dtype=np.float64)) / B
    return np.asarray(total, dtype=np.float32)


# --------------------------------------------------------------------------
# PJRT runner (jit built once per process)
# --------------------------------------------------------------------------

def make_runner(nc, n_cores=N_CORES):
    import jax
    from jax.sharding import Mesh, PartitionSpec
    from jax.experimental.shard_map import shard_map
    from concourse.bass2jax import (
        _bass_exec_p,
        install_neuronx_cc_hook,
        partition_id_tensor,
    )

    install_neuronx_cc_hook()
    partition_name = (
        nc.partition_id_tensor.name if nc.partition_id_tensor else None
    )

    in_names, out_names, out_avals, zero_outs = [], [], [], []
    for alloc in nc.m.functions[0].allocations:
        if not isinstance(alloc, mybir.MemoryLocationSet):
            continue
        name = alloc.memorylocations[0].name
        if alloc.kind == "ExternalInput":
            if name != partition_name:
                in_names.append(name)
        elif alloc.kind == "ExternalOutput":
            shape = tuple(alloc.tensor_shape)
            dtype = mybir.dt.np(alloc.dtype)
            out_names.append(name)
            out_avals.append(jax.core.ShapedArray(shape, dtype))
            zero_outs.append(np.zeros(shape, dtype))
    n_params = len(in_names)
    all_in_names = list(in_names) + list(out_names)
    if partition_name is not None:
        all_in_names.append(partition_name)

    def _body(*args):
        operands = list(args)
        if partition_name is not None:
            operands.append(partition_id_tensor())
        outs = _bass_exec_p.bind(
            *operands,
            out_avals=tuple(out_avals),
            in_names=tuple(all_in_names),
            out_names=tuple(out_names),
            lowering_input_output_aliases=(),
            sim_require_finite=True,
            sim_require_nnan=True,
            nc=nc,
        )
        return tuple(outs)

    devices = jax.devices()[:n_cores]
    mesh = Mesh(np.asarray(devices), ("core",))
    in_specs = (PartitionSpec("core"),) * (n_params + len(out_names))
    out_specs = (PartitionSpec("core"),) * len(out_names)
    jitted = jax.jit(
        shard_map(_body, mesh=mesh, in_specs=in_specs, out_specs=out_specs,
                  check_rep=False),
        keep_unused=True,
    )

    dev_cache = {}

    def run(in_maps, cache_key=None):
        import jax as _jax
        from jax.sharding import NamedSharding

        if cache_key is not None and cache_key in dev_cache:
            args = dev_cache[cache_key]
        else:
            concat_in = [
                np.concatenate(
                    [np.asarray(in_maps[c][n]) for c in range(n_cores)], axis=0
                )
                for n in in_names
            ]
            concat_zeros = [
                np.zeros((n_cores * z.shape[0], *z.shape[1:]), z.dtype)
                for z in zero_outs
            ]
            args = concat_in + concat_zeros
            if cache_key is not None:
                sh = NamedSharding(mesh, PartitionSpec("core"))
                args = [_jax.device_put(a, sh) for a in args]
                dev_cache[cache_key] = args
        outs = jitted(*args)
        _jax.block_until_ready(outs)
        return [
            {
                name: np.asarray(outs[i]).reshape(
                    n_cores, *out_avals[i].shape
                )[c]
                for i, name in enumerate(out_names)
            }
            for c in range(n_cores)
        ]

    return run


_CACHE = {}


def kernel(pred, target):
    if "run" not in _CACHE:
        _CACHE["run"] = make_runner(build_kernel(0))
    meta = _sorted_views(pred, target)
    results = _CACHE["run"](make_in_maps(pred, target))
    return postprocess(results, meta)


# revision 16
# speedup vs baseline: 1.1642x; 1.1642x over previous
"""ChamferLoss Trainium2 kernel (8 NeuronCores, bass/Tile) — banded version.

pred, target: [2, 16384, 3] fp32 -> scalar fp32
  d[b,n,m] = ||pred[b,n] - target[b,m]||
  out = mean(min_m d) + mean(min_n d)

Strategy: both point sets are sorted by norm on the host. Since
d(p,t) >= | ||p|| - ||t|| |, the nearest neighbour of a pred is (almost
always) close in *norm rank*, so each 128-pred block only computes d^2
against a 2048-wide window of rank-aligned targets (8x less work than
brute force). Exactness is restored on the host: for every point the
norm-gap to the first UNSEEN target/pred rank lower-bounds all unseen
distances; points whose windowed min exceeds that bound are recomputed
exactly in numpy (a few thousand on this distribution).

Sharding: core c = (batch b=c//4, pred-quarter q=c%4): 4096 sorted preds
x a 6144-wide sorted-target slice (lo = 4096q-960, indices clamped at the
array ends so edge windows degrade gracefully into duplicated targets).
Per core:
  - PE: d^2 tiles via one K=128 bf16 matmul per 512 targets. The 30
    augmented contraction rows (three-term bf16 splits of the coordinates
    and squared norms, so d^2 = p2 + t2 - 2 p.t accumulates in fp32 PSUM
    at ~fp32 accuracy) are replicated 4x; the 4x-scaled sum is undone by
    the ScalarE convert's scale.
  - ScalarE: PSUM fp32 -> SBUF fp16 conversion (scaled by BOOST/NREP).
  - VectorE: per block, one tensor_tensor min folds the 2048-wide window
    into the backward accumulator bacc[128, 6144], and one
    tensor_tensor_reduce (pairwise min of the window halves + min
    accumulator) produces the block's forward min column directly.
  - Tail: PE transposes bacc in 128-col chunks (4 per PSUM tile); VectorE
    reduce_min over [128, 4, 128] -> per-position min.
Host: maps window positions back to target ranks, mins across cores,
applies the gap-condition patch-up, then sqrt + means (O(N) work).
"""

import ml_dtypes
import numpy as np

import concourse.bass as bass
import concourse.tile as tile
from concourse import mybir

F32 = mybir.dt.float32
F16 = mybir.dt.float16
BF16 = mybir.dt.bfloat16

B = 2
N = 16384          # preds per batch
M = 16384          # targets per batch
NQ = N // 4        # preds per core
KA = 30            # base augmented contraction depth
NREP = 4           # replication count, 32-aligned for PE 32x32 tiling
K = 128            # padded contraction depth
NB = NQ // 128     # pred blocks per core (32)
W = 2048           # target window per pred block
WSPAN = 6144       # per-core target slice width (covers 128*31 + 2048 = 6016)
WCOV = 128 * (NB - 1) + W   # = 6016, highest window end within the slice
LO_OFF = -960      # slice start = 4096*q + LO_OFF (clamped indexing)
MM_N = 512         # matmul free dim (one PSUM bank)
N_CORES = 8
BOOST = 64.0       # pre-conversion scale: keeps tiny d^2 out of fp16
                   # subnormals (max d^2 ~ 300 * 64 still << fp16 max)
FBIG = 60000.0     # "+inf" for f16 min accumulators
FWD_MODE = "fold"  # "ts_accum" | "ttr" | "fold" forward-min strategy


# --------------------------------------------------------------------------
# Workaround: this walrus build accepts at most one sync-wait command per
# instruction. Hoist extra waits onto same-engine NoOps placed just before.
# --------------------------------------------------------------------------

def _split_sync_waits(nc):
    counter = 0
    for block in nc.m.functions[0].blocks:
        insts = block.instructions
        out = []
        changed = False
        for inst in insts:
            si = inst.sync_info
            if si is not None and si.on_wait and len(si.on_wait) > 1:
                waits = list(si.on_wait)
                for w in waits[:-1]:
                    counter += 1
                    out.append(
                        mybir.InstNoOp(
                            name=f"waitnop-{counter}",
                            engine=inst.engine,
                            sync_info=mybir.SyncInfo(on_wait=[w], on_update=[]),
                        )
                    )
                si.on_wait = waits[-1:]
                changed = True
            out.append(inst)
        if changed:
            block.instructions = out


def _patch_bass():
    if getattr(bass.Bass, "_split_waits_patched", False):
        return
    orig = bass.Bass.to_json_bytes

    def to_json_bytes(self, *a, **kw):
        _split_sync_waits(self)
        # populate .instr bytes for InstISA subclasses (tensor_tensor_reduce
        # etc.) — raw bass doesn't run this pass and walrus then fails with
        # "ISA wrong length" on the empty instr field
        mybir.codegen_inst_isa_subclasses(self)
        return orig(self, *a, **kw)

    bass.Bass.to_json_bytes = to_json_bytes
    bass.Bass._split_waits_patched = True


# --------------------------------------------------------------------------
# Kernel builder
# --------------------------------------------------------------------------

def build_kernel(n_loop: int = 0):
    """n_loop=0: production straight-line kernel. n_loop>0: wrap the main
    (idempotent) compute in a For_i loop for slope timing."""
    _patch_bass()
    nc = bass.Bass()
    paug_d = nc.dram_tensor("paug", [K, NQ], BF16, kind="ExternalInput")
    taug_d = nc.dram_tensor("taug", [K, WSPAN], BF16, kind="ExternalInput")
    fmin_d = nc.dram_tensor("fmin", [128, NB], F16, kind="ExternalOutput")
    bmin_d = nc.dram_tensor("bmin", [128, WSPAN // 128], F32,
                            kind="ExternalOutput")

    CVT_SCALE = BOOST  # each 32x32 PE tile sums a single aug replica

    with tile.TileContext(nc) as tc:
        with (
            tc.tile_pool(name="singles", bufs=1) as singles,
            tc.tile_pool(name="work", bufs=3) as work,
        ):
            paug = singles.tile([K, NQ], BF16)
            taug = singles.tile([K, WSPAN], BF16)
            bacc = singles.tile([128, WSPAN], F16)
            fslab = singles.tile([128, NB * 128], F16)
            fmin_sb = singles.tile([128, NB], F16)
            bmin_sb = singles.tile([128, WSPAN // 128], F32)

            nc.sync.dma_start(out=paug[:], in_=paug_d[:])
            for g in range(3):
                nc.sync.dma_start(
                    out=taug[:, g * 2048:(g + 1) * 2048],
                    in_=taug_d[:, g * 2048:(g + 1) * 2048],
                )

            ident = singles.tile([128, 128], F16)
            nc.gpsimd.memset(ident[:], 0.0)
            nc.gpsimd.affine_select(
                out=ident[:],
                in_=ident[:],
                compare_op=mybir.AluOpType.not_equal,
                fill=1.0,
                base=0,
                pattern=[[-1, 128]],
                channel_multiplier=1,
            )
            # backward accumulator starts at "+inf" (outside the timed loop;
            # the min-accumulation below is idempotent across loop iters)
            nc.vector.memset(bacc[:], FBIG)

            with tc.tile_pool(name="psum", bufs=2, space="PSUM") as psum:
                def tail_fold(t4):
                    # backward partition fold for 4 x 128 cols of bacc
                    tp = psum.tile([128, 512], F16, name=f"tp{t4}", tag="d2")
                    for u in range(4):
                        t = t4 * 4 + u
                        nc.tensor.transpose(
                            tp[:, u * 128:(u + 1) * 128],
                            bacc[:, t * 128:(t + 1) * 128],
                            ident[:],
                        )
                    nc.vector.tensor_reduce(
                        out=bmin_sb[:, t4 * 4:(t4 + 1) * 4],
                        in_=tp[:].rearrange("p (u f) -> p u f", u=4),
                        axis=mybir.AxisListType.X,
                        op=mybir.AluOpType.min,
                    )

                def main_compute():
                    for nb in range(NB):
                        d2 = psum.tile([128, W], F32, name=f"d2_{nb}",
                                       tag="d2")
                        # 16-way 32x32 PE tiling: row-tile r sees aug replica
                        # r and streams target chunk r; col-tile c computes
                        # pred sub-block c. One pack covers 128 preds x 2048
                        # targets; row tile r writes PSUM bank r.
                        for r in range(4):
                            for c in range(4):
                                nc.tensor.matmul(
                                    d2[32 * c:32 * c + 32,
                                       r * MM_N:(r + 1) * MM_N],
                                    paug[32 * r:32 * r + 32,
                                         nb * 128 + 32 * c:
                                         nb * 128 + 32 * c + 32],
                                    taug[32 * r:32 * r + 32,
                                         nb * 128 + r * MM_N:
                                         nb * 128 + (r + 1) * MM_N],
                                    start=True,
                                    stop=True,
                                    tile_position=(32 * r, 32 * c),
                                )
                        cvt = work.tile([128, W], F16, name=f"cvt{nb}",
                                        tag="cvt")
                        nc.scalar.activation(
                            out=cvt[:], in_=d2[:],
                            func=mybir.ActivationFunctionType.Copy,
                            scale=CVT_SCALE,
                        )
                        # backward: fold the window into the running min
                        nc.vector.tensor_tensor(
                            out=bacc[:, nb * 128:nb * 128 + W],
                            in0=bacc[:, nb * 128:nb * 128 + W],
                            in1=cvt[:],
                            op=mybir.AluOpType.min,
                        )
                        # forward: fold the window down to 128 wide into the
                        # slab; the cross-block reduce happens once at the end
                        fold = work.tile([128, W // 2], F16,
                                         name=f"fold{nb}", tag="fold")
                        nc.vector.tensor_tensor(
                            out=fold[:],
                            in0=cvt[:, 0:W // 2],
                            in1=cvt[:, W // 2:W],
                            op=mybir.AluOpType.min,
                        )
                        w = W // 4
                        while w > 128:
                            nc.vector.tensor_tensor(
                                out=fold[:, 0:w],
                                in0=fold[:, 0:w],
                                in1=fold[:, w:2 * w],
                                op=mybir.AluOpType.min,
                            )
                            w //= 2
                        nc.vector.tensor_tensor(
                            out=fslab[:, nb * 128:(nb + 1) * 128],
                            in0=fold[:, 0:128],
                            in1=fold[:, 128:256],
                            op=mybir.AluOpType.min,
                        )
                        # bacc cols [0, 128*nb) are final once block nb is
                        # done; fold them now so the tail overlaps compute
                        if nb >= 1 and nb * 128 % 512 == 0:
                            tail_fold(nb // 4 - 1)
                    # forward cross-column fold: [128, NB, 128] -> [128, NB]
                    # via log2 tensor_tensor halvings on 3D APs (keeps 2x)
                    w = 64
                    sl = fslab[:].rearrange("p (nb f) -> p nb f", nb=NB)
                    while w >= 1:
                        nc.vector.tensor_tensor(
                            out=sl[:, :, 0:w],
                            in0=sl[:, :, 0:w],
                            in1=sl[:, :, w:2 * w],
                            op=mybir.AluOpType.min,
                        )
                        w //= 2
                    nc.vector.tensor_copy(
                        fmin_sb[:],
                        sl[:, :, 0:1].rearrange("p nb f -> p (nb f)"),
                    )
                    for t4 in range(NB // 4 - 1, WSPAN // 512):
                        tail_fold(t4)

                if n_loop:
                    with tc.For_i(0, n_loop, 1):
                        main_compute()
                else:
                    main_compute()

            nc.sync.dma_start(out=fmin_d[:], in_=fmin_sb[:])
            nc.sync.dma_start(out=bmin_d[:], in_=bmin_sb[:])
    return nc


# --------------------------------------------------------------------------
# Host-side prep: augmented coordinate matrices. Each fp32 value is split
# into three bf16 terms (h + m + l reproduces the fp32 value to ~2^-24), so
# the expanded d^2 = p2 + t2 - 2 p.t keeps ~fp32-level absolute accuracy
# even for near-duplicate clouds where d^2 << |p|^2 (heavy cancellation).
# Cross terms keep the 8 products with magnitude >= 2^-25 (drop l*l);
# 30 rows total, replicated NREP=4 times and zero-padded to K=128.
# --------------------------------------------------------------------------

def _bf16(x):
    return x.astype(ml_dtypes.bfloat16)


def _split3(x):
    """fp32 array -> three bf16 arrays whose sum reproduces x to ~2^-24."""
    h = _bf16(x)
    r1 = x - h.astype(np.float32)
    m = _bf16(r1)
    l = _bf16(r1 - m.astype(np.float32))
    return h, m, l


def _aug_parts(coords):
    c = coords.astype(np.float32).T  # [3, n]
    n2 = c[0] * c[0] + c[1] * c[1] + c[2] * c[2]  # fp32, matches reference
    return _split3(c), _split3(n2)


def _replicate(base):
    # replicas at partitions 0/32/64/96 so each 32x32 PE row-tile sees one
    out = np.zeros((K, base.shape[1]), dtype=ml_dtypes.bfloat16)
    for r in range(NREP):
        out[r * 32:r * 32 + KA] = base
    return out


# (pred_term, target_term) index pairs for the 8 kept cross products
_CROSS = [(0, 0), (0, 1), (0, 2), (1, 0), (1, 1), (1, 2), (2, 0), (2, 1)]


def _aug_pred(coords):
    (ch, cm, cl), (n2h, n2m, n2l) = _aug_parts(coords)
    terms = [ch, cm, cl]
    base = np.zeros((KA, coords.shape[0]), dtype=ml_dtypes.bfloat16)
    for i, (pi, _) in enumerate(_CROSS):
        base[3 * i:3 * i + 3] = _bf16(-2.0 * terms[pi].astype(np.float32))
    base[24] = n2h
    base[25] = n2m
    base[26] = n2l
    base[27:30] = 1.0
    return _replicate(base)


def _aug_target(coords):
    (ch, cm, cl), (n2h, n2m, n2l) = _aug_parts(coords)
    terms = [ch, cm, cl]
    base = np.zeros((KA, coords.shape[0]), dtype=ml_dtypes.bfloat16)
    for i, (_, ti) in enumerate(_CROSS):
        base[3 * i:3 * i + 3] = terms[ti]
    base[24:27] = 1.0
    base[27] = n2h
    base[28] = n2m
    base[29] = n2l
    return _replicate(base)


def _sorted_views(pred, target):
    """Per batch: norm-sorted points + norms (the kernel's working order)."""
    meta = []
    for b in range(B):
        p = np.asarray(pred[b], np.float32)
        t = np.asarray(target[b], np.float32)
        pn = np.sqrt(np.sum(p * p, axis=1))
        tn = np.sqrt(np.sum(t * t, axis=1))
        po = np.argsort(pn, kind="stable")
        to = np.argsort(tn, kind="stable")
        meta.append({
            "ps": p[po], "ts": t[to],
            "psn": pn[po], "tsn": tn[to],
        })
    return meta


def make_in_maps(pred, target):
    meta = _sorted_views(pred, target)
    in_maps = []
    for b in range(B):
        ps, ts = meta[b]["ps"], meta[b]["ts"]
        taug_full = _aug_target(ts)
        for q in range(4):
            lo = 4096 * q + LO_OFF
            idx = np.clip(np.arange(lo, lo + WSPAN), 0, M - 1)
            in_maps.append({
                "paug": _aug_pred(ps[q * NQ:(q + 1) * NQ]),
                "taug": np.ascontiguousarray(taug_full[:, idx]),
            })
    # core order: c = b*4 + q
    return in_maps


# --------------------------------------------------------------------------
# Host post: map window positions back to ranks, min across cores, verify
# the norm-gap bound, recompute flagged points exactly, then sqrt + mean.
# --------------------------------------------------------------------------

def _exact_rows(pts, others):
    """Exact min distance from each row of pts[V,3] to others[M,3] (fp32,
    same formula as the reference)."""
    p2 = np.sum(pts * pts, axis=1, dtype=np.float32)[:, None]
    t2 = np.sum(others * others, axis=1, dtype=np.float32)[None, :]
    d2 = p2 + t2 - 2.0 * (pts @ others.T)
    return np.sqrt(np.maximum(d2.min(axis=1), 0.0))


def postprocess(results, meta):
    total = np.float64(0.0)
    pos = np.arange(WSPAN)
    i_min = np.maximum(0, -(-(pos - (W - 1)) // 128))   # ceil((pos-2047)/128)
    i_max = np.minimum(NB - 1, pos // 128)
    covered_pos = (i_min <= i_max) & (pos < WCOV)
    for b in range(B):
        mb = meta[b]
        psn, tsn = mb["psn"], mb["tsn"]
        ps, ts = mb["ps"], mb["ts"]

        fmin = np.full(N, np.inf, np.float32)
        bmin = np.full(M, np.inf, np.float32)
        pLo = np.full(M, N, np.int64)
        pHi = np.full(M, 0, np.int64)
        covL = np.full(N, 0, np.int64)
        covR = np.full(N, 0, np.int64)
        for q in range(4):
            r = results[b * 4 + q]
            lo = 4096 * q + LO_OFF
            # forward: fmin_sb[p, i] = window min for pred rank 4096q+128i+p
            f = np.asarray(r["fmin"]).T.reshape(-1)       # rank = 128*i + p
            fmin[q * NQ:(q + 1) * NQ] = f
            blk = np.arange(NQ) // 128
            covL[q * NQ:(q + 1) * NQ] = np.clip(lo + 128 * blk, 0, M)
            covR[q * NQ:(q + 1) * NQ] = np.clip(lo + 128 * blk + W, 0, M)
            # backward: bmin_sb[p, t] = min over preds for position 128t+p
            bm = np.asarray(r["bmin"]).T.reshape(-1)      # position
            ranks = np.clip(lo + pos, 0, M - 1)
            sel = covered_pos
            np.minimum.at(bmin, ranks[sel], bm[sel])
            np.minimum.at(pLo, ranks[sel], q * NQ + 128 * i_min[sel])
            np.maximum.at(pHi, ranks[sel], q * NQ + 128 * i_max[sel] + 128)

        fwd = np.sqrt(np.maximum(fmin * np.float32(1.0 / BOOST), 0.0,
                                 dtype=np.float32))
        bwd = np.sqrt(np.maximum(bmin * np.float32(1.0 / BOOST), 0.0,
                                 dtype=np.float32))

        # gap condition (forward): unseen targets are all below covL or at/
        # above covR in rank; their distance is >= the norm gap.
        gapL = np.where(covL > 0, psn - tsn[np.maximum(covL - 1, 0)], np.inf)
        gapR = np.where(covR < M, tsn[np.minimum(covR, M - 1)] - psn, np.inf)
        gap = np.minimum(np.maximum(gapL, 0.0), np.maximum(gapR, 0.0))
        bad = fwd * np.float32(1.002) + np.float32(1e-6) > gap
        if np.any(bad):
            fwd[bad] = _exact_rows(ps[bad], ts)

        # gap condition (backward)
        gapL = np.where(pLo > 0, tsn - psn[np.maximum(pLo - 1, 0)], np.inf)
        gapR = np.where(pHi < N, psn[np.minimum(pHi, N - 1)] - tsn, np.inf)
        gap = np.minimum(np.maximum(gapL, 0.0), np.maximum(gapR, 0.0))
        bad = bwd * np.float32(1.002) + np.float32(1e-6) > gap
        if np.any(bad):
            bwd[bad] = _exact_rows(ts[bad], ps)

        total += (fwd.mean(dtype=np.float64) + bwd.mean(dtype=np.float64)) / B
    return np.asarray(total, dtype=np.float32)


# --------------------------------------------------------------------------
# PJRT runner (jit built once per process)
# --------------------------------------------------------------------------

def make_runner(nc, n_cores=N_CORES):
    import jax
    from jax.sharding import Mesh, PartitionSpec
    from jax.experimental.shard_map import shard_map
    from concourse.bass2jax import (
        _bass_exec_p,
        install_neuronx_cc_hook,
        partition_id_tensor,
    )

    install_neuronx_cc_hook()
    partition_name = (
        nc.partition_id_tensor.name if nc.partition_id_tensor else None
    )

    in_names, out_names, out_avals, zero_outs = [], [], [], []
    for alloc in nc.m.functions[0].allocations:
        if not isinstance(alloc, mybir.MemoryLocationSet):
            continue
        name = alloc.memorylocations[0].name
        if alloc.kind == "ExternalInput":
            if name != partition_name:
                in_names.append(name)
        elif alloc.kind == "ExternalOutput":
            shape = tuple(alloc.tensor_shape)
            dtype = mybir.dt.np(alloc.dtype)
            out_names.append(name)
            out_avals.append(jax.core.ShapedArray(shape, dtype))
            zero_outs.append(np.zeros(shape, dtype))
    n_params = len(in_names)
    all_in_names = list(in_names) + list(out_names)
    if partition_name is not None:
        all_in_names.append(partition_name)

    def _body(*args):
        operands = list(args)
        if partition_name is not None:
            operands.append(partition_id_tensor())
        outs = _bass_exec_p.bind(
            *operands,
            out_avals=tuple(out_avals),
            in_names=tuple(all_in_names),
            out_names=tuple(out_names),
            lowering_input_output_aliases=(),
            sim_require_finite=True,
            sim_require_nnan=True,
            nc=nc,
        )
        return tuple(outs)

    devices = jax.devices()[:n_cores]
    mesh = Mesh(np.asarray(devices), ("core",))
    in_specs = (PartitionSpec("core"),) * (n_params + len(out_names))
    out_specs = (PartitionSpec("core"),) * len(out_names)
    jitted = jax.jit(
        shard_map(_body, mesh=mesh, in_specs=in_specs, out_specs=out_specs,
                  check_rep=False),
        keep_unused=True,
    )

    dev_cache = {}

    def run(in_maps, cache_key=None):
        import jax as _jax
        from jax.sharding import NamedSharding

        if cache_key is not None and cache_key in dev_cache:
            args = dev_cache[cache_key]
        else:
            concat_in = [
                np.concatenate(
                    [np.asarray(in_maps[c][n]) for c in range(n_cores)], axis=0
                )
                for n in in_names
            ]
            concat_zeros = [
                np.zeros((n_cores * z.shape[0], *z.shape[1:]), z.dtype)
                for z in zero_outs
            ]
            args = concat_in + concat_zeros
            if cache_key is not None:
                sh = NamedSharding(mesh, PartitionSpec("core"))
                args = [_jax.device_put(a, sh) for a in args]
                dev_cache[cache_key] = args
        outs = jitted(*args)
        _jax.block_until_ready(outs)
        return [
            {
                name: np.asarray(outs[i]).reshape(
                    n_cores, *out_avals[i].shape
                )[c]
                for i, name in enumerate(out_names)
            }
            for c in range(n_cores)
        ]

    return run


_CACHE = {}


def kernel(pred, target):
    if "run" not in _CACHE:
        _CACHE["run"] = make_runner(build_kernel(0))
    meta = _sorted_views(pred, target)
    results = _CACHE["run"](make_in_maps(pred, target))
    return postprocess(results, meta)


# revision 20
# speedup vs baseline: 2.1101x; 1.8125x over previous
"""ChamferLoss Trainium2 kernel (8 NeuronCores, bass/Tile) — banded version.

pred, target: [2, 16384, 3] fp32 -> scalar fp32
  d[b,n,m] = ||pred[b,n] - target[b,m]||
  out = mean(min_m d) + mean(min_n d)

Strategy: both point sets are sorted by norm on the host. Since
d(p,t) >= | ||p|| - ||t|| |, the nearest neighbour of a pred is (almost
always) close in *norm rank*, so each 128-pred block only computes d^2
against a 2048-wide window of rank-aligned targets (8x less work than
brute force). Exactness is restored on the host: for every point the
norm-gap to the first UNSEEN target/pred rank lower-bounds all unseen
distances; points whose windowed min exceeds that bound are recomputed
exactly in numpy (a few thousand on this distribution).

Sharding: core c = (batch b=c//4, pred-quarter q=c%4): 4096 sorted preds
x a 6144-wide sorted-target slice (lo = 4096q-960, indices clamped at the
array ends so edge windows degrade gracefully into duplicated targets).
Per core:
  - PE: d^2 tiles via one K=128 bf16 matmul per 512 targets. The 30
    augmented contraction rows (three-term bf16 splits of the coordinates
    and squared norms, so d^2 = p2 + t2 - 2 p.t accumulates in fp32 PSUM
    at ~fp32 accuracy) are replicated 4x; the 4x-scaled sum is undone by
    the ScalarE convert's scale.
  - ScalarE: PSUM fp32 -> SBUF fp16 conversion (scaled by BOOST/NREP).
  - VectorE: per block, one tensor_tensor min folds the 2048-wide window
    into the backward accumulator bacc[128, 6144], and one
    tensor_tensor_reduce (pairwise min of the window halves + min
    accumulator) produces the block's forward min column directly.
  - Tail: PE transposes bacc in 128-col chunks (4 per PSUM tile); VectorE
    reduce_min over [128, 4, 128] -> per-position min.
Host: maps window positions back to target ranks, mins across cores,
applies the gap-condition patch-up, then sqrt + means (O(N) work).
"""

import ml_dtypes
import numpy as np

import concourse.bass as bass
import concourse.tile as tile
from concourse import mybir

F32 = mybir.dt.float32
F16 = mybir.dt.float16
BF16 = mybir.dt.bfloat16

B = 2
N = 16384          # preds per batch
M = 16384          # targets per batch
NQ = N // 4        # preds per core
KA = 30            # base augmented contraction depth
NREP = 4           # replication count, 32-aligned for PE 32x32 tiling
K = 128            # padded contraction depth
NB = NQ // 128     # pred blocks per core (32)
W = 1024           # target window per pred block (power of two >= 512)
WCOV = 128 * (NB - 1) + W   # highest window end within the slice
WSPAN = -(-WCOV // 512) * 512   # per-core target slice width (512-aligned)
LO_OFF = -(W // 2 - 64)  # slice start = 4096*q + LO_OFF (clamped indexing)
CHUNK = W // 4     # matmul free dim per PE row-tile (PSUM bank r holds
                   # the block's target chunk r in its first CHUNK cols)
N_CORES = 8
BOOST = 64.0       # pre-conversion scale: keeps tiny d^2 out of fp16
                   # subnormals (max d^2 ~ 300 * 64 still << fp16 max)
FBIG = 60000.0     # "+inf" for f16 min accumulators
FWD_MODE = "fold"  # "ts_accum" | "ttr" | "fold" forward-min strategy


# --------------------------------------------------------------------------
# Workaround: this walrus build accepts at most one sync-wait command per
# instruction. Hoist extra waits onto same-engine NoOps placed just before.
# --------------------------------------------------------------------------

def _split_sync_waits(nc):
    counter = 0
    for block in nc.m.functions[0].blocks:
        insts = block.instructions
        out = []
        changed = False
        for inst in insts:
            si = inst.sync_info
            if si is not None and si.on_wait and len(si.on_wait) > 1:
                waits = list(si.on_wait)
                for w in waits[:-1]:
                    counter += 1
                    out.append(
                        mybir.InstNoOp(
                            name=f"waitnop-{counter}",
                            engine=inst.engine,
                            sync_info=mybir.SyncInfo(on_wait=[w], on_update=[]),
                        )
                    )
                si.on_wait = waits[-1:]
                changed = True
            out.append(inst)
        if changed:
            block.instructions = out


def _patch_bass():
    if getattr(bass.Bass, "_split_waits_patched", False):
        return
    orig = bass.Bass.to_json_bytes

    def to_json_bytes(self, *a, **kw):
        _split_sync_waits(self)
        # populate .instr bytes for InstISA subclasses (tensor_tensor_reduce
        # etc.) — raw bass doesn't run this pass and walrus then fails with
        # "ISA wrong length" on the empty instr field
        mybir.codegen_inst_isa_subclasses(self)
        return orig(self, *a, **kw)

    bass.Bass.to_json_bytes = to_json_bytes
    bass.Bass._split_waits_patched = True


# --------------------------------------------------------------------------
# Kernel builder
# --------------------------------------------------------------------------

def build_kernel(n_loop: int = 0):
    """n_loop=0: production straight-line kernel. n_loop>0: wrap the main
    (idempotent) compute in a For_i loop for slope timing."""
    _patch_bass()
    nc = bass.Bass()
    paug_d = nc.dram_tensor("paug", [K, NQ], BF16, kind="ExternalInput")
    taug_d = nc.dram_tensor("taug", [K, WSPAN], BF16, kind="ExternalInput")
    fmin_d = nc.dram_tensor("fmin", [128, NB], F16, kind="ExternalOutput")
    bmin_d = nc.dram_tensor("bmin", [128, WSPAN // 128], F32,
                            kind="ExternalOutput")

    CVT_SCALE = BOOST  # each 32x32 PE tile sums a single aug replica

    with tile.TileContext(nc) as tc:
        with (
            tc.tile_pool(name="singles", bufs=1) as singles,
            tc.tile_pool(name="work", bufs=3) as work,
        ):
            paug = singles.tile([K, NQ], BF16)
            taug = singles.tile([K, WSPAN], BF16)
            bacc = singles.tile([128, WSPAN], F16)
            fslab = singles.tile([128, NB * 128], F16)
            fmin_sb = singles.tile([128, NB], F16)
            bmin_sb = singles.tile([128, WSPAN // 128], F32)

            nc.sync.dma_start(out=paug[:], in_=paug_d[:])
            for g in range(0, WSPAN, 2048):
                e = min(g + 2048, WSPAN)
                nc.sync.dma_start(
                    out=taug[:, g:e],
                    in_=taug_d[:, g:e],
                )

            ident = singles.tile([128, 128], F16)
            nc.gpsimd.memset(ident[:], 0.0)
            nc.gpsimd.affine_select(
                out=ident[:],
                in_=ident[:],
                compare_op=mybir.AluOpType.not_equal,
                fill=1.0,
                base=0,
                pattern=[[-1, 128]],
                channel_multiplier=1,
            )
            # backward accumulator starts at "+inf" (outside the timed loop;
            # the min-accumulation below is idempotent across loop iters)
            nc.vector.memset(bacc[:], FBIG)

            with tc.tile_pool(name="psum", bufs=2, space="PSUM") as psum:
                def tail_fold(t4):
                    # backward partition fold for 4 x 128 cols of bacc
                    tp = psum.tile([128, 512], F16, name=f"tp{t4}", tag="d2")
                    for u in range(4):
                        t = t4 * 4 + u
                        nc.tensor.transpose(
                            tp[:, u * 128:(u + 1) * 128],
                            bacc[:, t * 128:(t + 1) * 128],
                            ident[:],
                        )
                    nc.vector.tensor_reduce(
                        out=bmin_sb[:, t4 * 4:(t4 + 1) * 4],
                        in_=tp[:].rearrange("p (u f) -> p u f", u=4),
                        axis=mybir.AxisListType.X,
                        op=mybir.AluOpType.min,
                    )

                def main_compute():
                    for nb in range(NB):
                        d2 = psum.tile([128, 2048], F32, name=f"d2_{nb}",
                                       tag="d2")
                        # 16-way 32x32 PE tiling: row-tile r sees aug replica
                        # r and streams target chunk r; col-tile c computes
                        # pred sub-block c. One pack covers 128 preds x W
                        # targets; row tile r writes PSUM bank r (first
                        # CHUNK cols).
                        for r in range(4):
                            for c in range(4):
                                nc.tensor.matmul(
                                    d2[32 * c:32 * c + 32,
                                       r * 512:r * 512 + CHUNK],
                                    paug[32 * r:32 * r + 32,
                                         nb * 128 + 32 * c:
                                         nb * 128 + 32 * c + 32],
                                    taug[32 * r:32 * r + 32,
                                         nb * 128 + r * CHUNK:
                                         nb * 128 + (r + 1) * CHUNK],
                                    start=True,
                                    stop=True,
                                    tile_position=(32 * r, 32 * c),
                                )
                        cvt = work.tile([128, W], F16, name=f"cvt{nb}",
                                        tag="cvt")
                        nc.scalar.activation(
                            out=cvt[:].rearrange("p (r g) -> p r g", r=4),
                            in_=d2[:].rearrange("p (r g) -> p r g",
                                                r=4)[:, :, 0:CHUNK],
                            func=mybir.ActivationFunctionType.Copy,
                            scale=CVT_SCALE,
                        )
                        # backward: fold the window into the running min
                        nc.vector.tensor_tensor(
                            out=bacc[:, nb * 128:nb * 128 + W],
                            in0=bacc[:, nb * 128:nb * 128 + W],
                            in1=cvt[:],
                            op=mybir.AluOpType.min,
                        )
                        # forward: fold the window down to 128 wide into the
                        # slab; the cross-block reduce happens once at the end
                        fold = work.tile([128, W // 2], F16,
                                         name=f"fold{nb}", tag="fold")
                        nc.vector.tensor_tensor(
                            out=fold[:],
                            in0=cvt[:, 0:W // 2],
                            in1=cvt[:, W // 2:W],
                            op=mybir.AluOpType.min,
                        )
                        w = W // 4
                        while w > 128:
                            nc.vector.tensor_tensor(
                                out=fold[:, 0:w],
                                in0=fold[:, 0:w],
                                in1=fold[:, w:2 * w],
                                op=mybir.AluOpType.min,
                            )
                            w //= 2
                        nc.vector.tensor_tensor(
                            out=fslab[:, nb * 128:(nb + 1) * 128],
                            in0=fold[:, 0:128],
                            in1=fold[:, 128:256],
                            op=mybir.AluOpType.min,
                        )
                        # bacc cols [0, 128*nb) are final once block nb is
                        # done; fold them now so the tail overlaps compute
                        if nb >= 1 and nb * 128 % 512 == 0:
                            tail_fold(nb // 4 - 1)
                    # forward cross-column fold: [128, NB, 128] -> [128, NB]
                    # via log2 tensor_tensor halvings on 3D APs (keeps 2x)
                    w = 64
                    sl = fslab[:].rearrange("p (nb f) -> p nb f", nb=NB)
                    while w >= 1:
                        nc.vector.tensor_tensor(
                            out=sl[:, :, 0:w],
                            in0=sl[:, :, 0:w],
                            in1=sl[:, :, w:2 * w],
                            op=mybir.AluOpType.min,
                        )
                        w //= 2
                    nc.vector.tensor_copy(
                        fmin_sb[:],
                        sl[:, :, 0:1].rearrange("p nb f -> p (nb f)"),
                    )
                    for t4 in range(NB // 4 - 1, WSPAN // 512):
                        tail_fold(t4)

                if n_loop:
                    with tc.For_i(0, n_loop, 1):
                        main_compute()
                else:
                    main_compute()

            nc.sync.dma_start(out=fmin_d[:], in_=fmin_sb[:])
            nc.sync.dma_start(out=bmin_d[:], in_=bmin_sb[:])
    return nc


# --------------------------------------------------------------------------
# Host-side prep: augmented coordinate matrices. Each fp32 value is split
# into three bf16 terms (h + m + l reproduces the fp32 value to ~2^-24), so
# the expanded d^2 = p2 + t2 - 2 p.t keeps ~fp32-level absolute accuracy
# even for near-duplicate clouds where d^2 << |p|^2 (heavy cancellation).
# Cross terms keep the 8 products with magnitude >= 2^-25 (drop l*l);
# 30 rows total, replicated NREP=4 times and zero-padded to K=128.
# --------------------------------------------------------------------------

def _bf16(x):
    return x.astype(ml_dtypes.bfloat16)


def _split3(x):
    """fp32 array -> three bf16 arrays whose sum reproduces x to ~2^-24."""
    h = _bf16(x)
    r1 = x - h.astype(np.float32)
    m = _bf16(r1)
    l = _bf16(r1 - m.astype(np.float32))
    return h, m, l


def _aug_parts(coords):
    c = coords.astype(np.float32).T  # [3, n]
    n2 = c[0] * c[0] + c[1] * c[1] + c[2] * c[2]  # fp32, matches reference
    return _split3(c), _split3(n2)


def _replicate(base):
    # replicas at partitions 0/32/64/96 so each 32x32 PE row-tile sees one
    out = np.zeros((K, base.shape[1]), dtype=ml_dtypes.bfloat16)
    for r in range(NREP):
        out[r * 32:r * 32 + KA] = base
    return out


# (pred_term, target_term) index pairs for the 8 kept cross products
_CROSS = [(0, 0), (0, 1), (0, 2), (1, 0), (1, 1), (1, 2), (2, 0), (2, 1)]


def _aug_pred(coords):
    (ch, cm, cl), (n2h, n2m, n2l) = _aug_parts(coords)
    terms = [ch, cm, cl]
    base = np.zeros((KA, coords.shape[0]), dtype=ml_dtypes.bfloat16)
    for i, (pi, _) in enumerate(_CROSS):
        base[3 * i:3 * i + 3] = _bf16(-2.0 * terms[pi].astype(np.float32))
    base[24] = n2h
    base[25] = n2m
    base[26] = n2l
    base[27:30] = 1.0
    return _replicate(base)


def _aug_target(coords):
    (ch, cm, cl), (n2h, n2m, n2l) = _aug_parts(coords)
    terms = [ch, cm, cl]
    base = np.zeros((KA, coords.shape[0]), dtype=ml_dtypes.bfloat16)
    for i, (_, ti) in enumerate(_CROSS):
        base[3 * i:3 * i + 3] = terms[ti]
    base[24:27] = 1.0
    base[27] = n2h
    base[28] = n2m
    base[29] = n2l
    return _replicate(base)


def _sorted_views(pred, target):
    """Per batch: norm-sorted points + norms (the kernel's working order)."""
    meta = []
    for b in range(B):
        p = np.asarray(pred[b], np.float32)
        t = np.asarray(target[b], np.float32)
        pn = np.sqrt(np.sum(p * p, axis=1))
        tn = np.sqrt(np.sum(t * t, axis=1))
        po = np.argsort(pn, kind="stable")
        to = np.argsort(tn, kind="stable")
        meta.append({
            "ps": p[po], "ts": t[to],
            "psn": pn[po], "tsn": tn[to],
        })
    return meta


def make_in_maps(pred, target):
    meta = _sorted_views(pred, target)
    in_maps = []
    for b in range(B):
        ps, ts = meta[b]["ps"], meta[b]["ts"]
        taug_full = _aug_target(ts)
        for q in range(4):
            lo = 4096 * q + LO_OFF
            idx = np.clip(np.arange(lo, lo + WSPAN), 0, M - 1)
            in_maps.append({
                "paug": _aug_pred(ps[q * NQ:(q + 1) * NQ]),
                "taug": np.ascontiguousarray(taug_full[:, idx]),
            })
    # core order: c = b*4 + q
    return in_maps


# --------------------------------------------------------------------------
# Host post: map window positions back to ranks, min across cores, verify
# the norm-gap bound, recompute flagged points exactly, then sqrt + mean.
# --------------------------------------------------------------------------

def _exact_rows(pts, others):
    """Exact min distance from each row of pts[V,3] to others[M,3] (fp32,
    same formula as the reference)."""
    t2 = np.sum(others * others, axis=1, dtype=np.float32)[None, :]
    out = np.empty(len(pts), np.float32)
    for s in range(0, len(pts), 4096):
        p = pts[s:s + 4096]
        p2 = np.sum(p * p, axis=1, dtype=np.float32)[:, None]
        d2 = p2 + t2 - 2.0 * (p @ others.T)
        out[s:s + 4096] = np.sqrt(np.maximum(d2.min(axis=1), 0.0))
    return out


def postprocess(results, meta):
    total = np.float64(0.0)
    pos = np.arange(WSPAN)
    i_min = np.maximum(0, -(-(pos - (W - 1)) // 128))   # ceil((pos-2047)/128)
    i_max = np.minimum(NB - 1, pos // 128)
    covered_pos = (i_min <= i_max) & (pos < WCOV)
    for b in range(B):
        mb = meta[b]
        psn, tsn = mb["psn"], mb["tsn"]
        ps, ts = mb["ps"], mb["ts"]

        fmin = np.full(N, np.inf, np.float32)
        bmin = np.full(M, np.inf, np.float32)
        pLo = np.full(M, N, np.int64)
        pHi = np.full(M, 0, np.int64)
        covL = np.full(N, 0, np.int64)
        covR = np.full(N, 0, np.int64)
        for q in range(4):
            r = results[b * 4 + q]
            lo = 4096 * q + LO_OFF
            # forward: fmin_sb[p, i] = window min for pred rank 4096q+128i+p
            f = np.asarray(r["fmin"]).T.reshape(-1)       # rank = 128*i + p
            fmin[q * NQ:(q + 1) * NQ] = f
            blk = np.arange(NQ) // 128
            covL[q * NQ:(q + 1) * NQ] = np.clip(lo + 128 * blk, 0, M)
            covR[q * NQ:(q + 1) * NQ] = np.clip(lo + 128 * blk + W, 0, M)
            # backward: bmin_sb[p, t] = min over preds for position 128t+p
            bm = np.asarray(r["bmin"]).T.reshape(-1)      # position
            ranks = np.clip(lo + pos, 0, M - 1)
            sel = covered_pos
            np.minimum.at(bmin, ranks[sel], bm[sel])
            np.minimum.at(pLo, ranks[sel], q * NQ + 128 * i_min[sel])
            np.maximum.at(pHi, ranks[sel], q * NQ + 128 * i_max[sel] + 128)

        fwd = np.sqrt(np.maximum(fmin * np.float32(1.0 / BOOST), 0.0,
                                 dtype=np.float32))
        bwd = np.sqrt(np.maximum(bmin * np.float32(1.0 / BOOST), 0.0,
                                 dtype=np.float32))

        # gap condition (forward): unseen targets are all below covL or at/
        # above covR in rank; their distance is >= the norm gap.
        gapL = np.where(covL > 0, psn - tsn[np.maximum(covL - 1, 0)], np.inf)
        gapR = np.where(covR < M, tsn[np.minimum(covR, M - 1)] - psn, np.inf)
        gap = np.minimum(np.maximum(gapL, 0.0), np.maximum(gapR, 0.0))
        bad = fwd * np.float32(1.002) + np.float32(1e-6) > gap
        if np.any(bad):
            fwd[bad] = _exact_rows(ps[bad], ts)

        # gap condition (backward)
        gapL = np.where(pLo > 0, tsn - psn[np.maximum(pLo - 1, 0)], np.inf)
        gapR = np.where(pHi < N, psn[np.minimum(pHi, N - 1)] - tsn, np.inf)
        gap = np.minimum(np.maximum(gapL, 0.0), np.maximum(gapR, 0.0))
        bad = bwd * np.float32(1.002) + np.float32(1e-6) > gap
        if np.any(bad):
            bwd[bad] = _exact_rows(ts[bad], ps)

        total += (fwd.mean(dtype=np.float64) + bwd.mean(dtype=np.float64)) / B
    return np.asarray(total, dtype=np.float32)


# --------------------------------------------------------------------------
# PJRT runner (jit built once per process)
# --------------------------------------------------------------------------

def make_runner(nc, n_cores=N_CORES):
    import jax
    from jax.sharding import Mesh, PartitionSpec
    from jax.experimental.shard_map import shard_map
    from concourse.bass2jax import (
        _bass_exec_p,
        install_neuronx_cc_hook,
        partition_id_tensor,
    )

    install_neuronx_cc_hook()
    partition_name = (
        nc.partition_id_tensor.name if nc.partition_id_tensor else None
    )

    in_names, out_names, out_avals, zero_outs = [], [], [], []
    for alloc in nc.m.functions[0].allocations:
        if not isinstance(alloc, mybir.MemoryLocationSet):
            continue
        name = alloc.memorylocations[0].name
        if alloc.kind == "ExternalInput":
            if name != partition_name:
                in_names.append(name)
        elif alloc.kind == "ExternalOutput":
            shape = tuple(alloc.tensor_shape)
            dtype = mybir.dt.np(alloc.dtype)
            out_names.append(name)
            out_avals.append(jax.core.ShapedArray(shape, dtype))
            zero_outs.append(np.zeros(shape, dtype))
    n_params = len(in_names)
    all_in_names = list(in_names) + list(out_names)
    if partition_name is not None:
        all_in_names.append(partition_name)

    def _body(*args):
        operands = list(args)
        if partition_name is not None:
            operands.append(partition_id_tensor())
        outs = _bass_exec_p.bind(
            *operands,
            out_avals=tuple(out_avals),
            in_names=tuple(all_in_names),
            out_names=tuple(out_names),
            lowering_input_output_aliases=(),
            sim_require_finite=True,
            sim_require_nnan=True,
            nc=nc,
        )
        return tuple(outs)

    devices = jax.devices()[:n_cores]
    mesh = Mesh(np.asarray(devices), ("core",))
    in_specs = (PartitionSpec("core"),) * (n_params + len(out_names))
    out_specs = (PartitionSpec("core"),) * len(out_names)
    jitted = jax.jit(
        shard_map(_body, mesh=mesh, in_specs=in_specs, out_specs=out_specs,
                  check_rep=False),
        keep_unused=True,
    )

    dev_cache = {}

    def run(in_maps, cache_key=None):
        import jax as _jax
        from jax.sharding import NamedSharding

        if cache_key is not None and cache_key in dev_cache:
            args = dev_cache[cache_key]
        else:
            concat_in = [
                np.concatenate(
                    [np.asarray(in_maps[c][n]) for c in range(n_cores)], axis=0
                )
                for n in in_names
            ]
            concat_zeros = [
                np.zeros((n_cores * z.shape[0], *z.shape[1:]), z.dtype)
                for z in zero_outs
            ]
            args = concat_in + concat_zeros
            if cache_key is not None:
                sh = NamedSharding(mesh, PartitionSpec("core"))
                args = [_jax.device_put(a, sh) for a in args]
                dev_cache[cache_key] = args
        outs = jitted(*args)
        _jax.block_until_ready(outs)
        return [
            {
                name: np.asarray(outs[i]).reshape(
                    n_cores, *out_avals[i].shape
                )[c]
                for i, name in enumerate(out_names)
            }
            for c in range(n_cores)
        ]

    return run


_CACHE = {}


def kernel(pred, target):
    if "run" not in _CACHE:
        _CACHE["run"] = make_runner(build_kernel(0))
    meta = _sorted_views(pred, target)
    results = _CACHE["run"](make_in_maps(pred, target))
    return postprocess(results, meta)
